# revision 19
# baseline (speedup 1.0000x reference)
"""Trainium2 Bass kernel for nn_AttnBlock (B=1, C=128, H=32, W=128, 8 heads).

Sharding: one attention head per NeuronCore (8 heads / 8 cores). Each core
computes its head's attention over L=4096 positions and the final W-axis
projection for its 16-channel output slab. Host gathers 8 slabs.

v2 design (vs the flash baseline):
  * S^T via the rank-16 factor-through-weights trick: G = (4*Wk^T Wq) @ x is
    computed once on-device (128-deep contractions), then every S^T tile is
    x_tile^T @ G_chunk -- no q/k tensors, one evacuation (G) instead of two.
  * q-bias folded EXACTLY into a per-key reweighting of V: softmax(q_i.k_j +
    bq.k_j + const_i) => multiply [v_j|1] by w_j = exp(4*bq.k_j). The bias
    row 4*bq.k_j is produced as an extra column of the v matmul and exp'd on
    ScalarE; k-bias and bq.bk terms cancel in softmax exactly.
  * exp tiles (128 x 1536) split between ScalarE (exact exp -> fp8e4) and
    VectorE (Schraudolph int-bit trick -> uint8 saturating -> fp8e4 bits;
    negative-bit underflow saturates to 0.0 which is the correct flush).
  * A@V in fp8e4 with perf_mode=DoubleRow: one matmul contracts TWO j-tiles
    ([128, 2, 17] weights x [128, 2, 512] moving), with the softmax
    denominator as a 17th weight column (ones*w_j).
  * epilogue: transpose via idmatmul, batched reciprocal + broadcast-multiply
    normalize, W-axis projection in bf16.
"""

import math as _math

import numpy as np

N_CORES = 8
C = 128
H = 32
W = 128
L = H * W  # 4096
F = 8  # heads
D = 16  # head dim
CHUNK = 512
NCHUNK = L // CHUNK  # 8
NJT = L // 128  # 32 j-tiles
SHIFT = 2.5  # global exp shift for fp8 range (cancels in softmax)
A8 = 8.0 / _math.log(2.0)  # Schraudolph scale for e4m3 bits
B8P = (56.0 - 0.5) - SHIFT * A8  # magic + shift folded
CB_W = 20  # f32 cblob: idpad (17,18) | negshift col
# bf16 blob: wpbf 0:128 | wvb 128:160 | gw 160:288 | row0: bp512 288:800,
# ones16 800:816, bv32 816:848, ones128 848:976
BB_W = 976

# cost-model constants for build-time ACT/DVE load balancing (ns)
_ACT_CY = 1e9 / 1.2e9
_DVE_CY = 1e9 / 0.96e9


def _act_cost(fd):
    return (fd + 222) * _ACT_CY


def _dve_cost(fd, psum=True):
    return (fd + (120 if psum else 58)) * _DVE_CY


_CACHE = {}


def _build():
    import concourse.tile as tile
    from concourse import bacc, mybir

    f32 = mybir.dt.float32
    bf16 = mybir.dt.bfloat16
    fp8 = mybir.dt.float8e4
    u8 = mybir.dt.uint8
    Exp = mybir.ActivationFunctionType.Exp
    DR = mybir.MatmulPerfMode.DoubleRow

    nc = bacc.Bacc("TRN2", target_bir_lowering=False, debug=False)

    x_d = nc.dram_tensor("x_cl", [C, L], bf16, kind="ExternalInput").ap()
    cb_d = nc.dram_tensor("cblob", [C, CB_W], f32, kind="ExternalInput").ap()
    bb_d = nc.dram_tensor("bblob", [C, BB_W], bf16, kind="ExternalInput").ap()
    out_d = nc.dram_tensor("out", [D, L], f32, kind="ExternalOutput").ap()

    # build-time engine load (ns) for balancing flexible work
    load = {"act": 1283.0, "dve": 0.0}  # act table load charged up front

    def pick_engine():
        return "act" if load["act"] <= load["dve"] else "dve"

    with tile.TileContext(nc) as tc:
        with (
            tc.tile_pool(name="consts", bufs=1) as consts,
            tc.tile_pool(name="accsb", bufs=2) as accsbp,
            tc.tile_pool(name="episb", bufs=4) as episb,
        ):
            cb = consts.tile([C, CB_W], f32)
            idpad = cb[0:17, 0:18]
            negshift = cb[:, 18:19]
            bb = consts.tile([C, BB_W], bf16)
            wpbf = bb[:, 0:128]
            wvb = bb[:, 128:160]
            gw = bb[:, 160:288]
            bp512 = bb[0:1, 288:800]
            ones16 = bb[0:1, 800:816]
            bv32 = bb[0:1, 816:848]
            ones128row = bb[0:1, 848:976]

            x_sb = consts.tile([C, L], bf16)

            def dma_x(cch, q):
                q.dma_start(
                    out=x_sb[:, cch * CHUNK : (cch + 1) * CHUNK],
                    in_=x_d[:, cch * CHUNK : (cch + 1) * CHUNK],
                )

            dma_x(0, nc.sync)
            dma_x(1, nc.gpsimd)
            nc.sync.dma_start(out=bb, in_=bb_d)
            nc.scalar.dma_start(out=cb, in_=cb_d)
            dma_x(2, nc.scalar)
            dma_x(3, nc.gpsimd)
            dma_x(4, nc.sync)
            dma_x(5, nc.gpsimd)
            dma_x(6, nc.scalar)
            dma_x(7, nc.gpsimd)

            g_sb = consts.tile([C, L], bf16)
            et = consts.tile([C, NJT, CHUNK], fp8)
            v_sb = consts.tile([C, NJT, 32], fp8)
            wexp = consts.tile([C, NJT], f32)

            with (
                tc.tile_pool(name="ps_s", bufs=3, space="PSUM") as ps_s,
                tc.tile_pool(name="ps_acc", bufs=1, space="PSUM") as ps_acc,
                tc.tile_pool(name="ps_epi", bufs=1, space="PSUM") as ps_epi,
            ):
                # warm the ACT exp table immediately (no DMA dependency)
                dummy = episb.tile([1, 2], f32, tag="dummy")
                nc.gpsimd.memset(dummy[:], 0.5)
                nc.scalar.activation(out=dummy[:], in_=dummy[:], func=Exp)

                # ---- G = (4 Wk^T Wq) @ x and v-tile helpers; slice 0
                # upfront, the rest interleaved into chunk 0's schedule ----
                def emit_g_slice(s0, split=False):
                    gps = ps_s.tile([C, 2, CHUNK], f32, tag="squad", name=f"gps{s0}")
                    for t in range(2):
                        sl = slice((s0 + t) * CHUNK, (s0 + t + 1) * CHUNK)
                        nc.tensor.matmul(
                            gps[:, t, :], gw, x_sb[:, sl], start=True, stop=True
                        )
                        if split:
                            nc.vector.tensor_copy(
                                g_sb[:, (s0 + t) * CHUNK : (s0 + t + 1) * CHUNK],
                                gps[:, t, :],
                            )
                            load["dve"] += _dve_cost(CHUNK)
                    if not split:
                        nc.vector.tensor_copy(
                            g_sb[:, s0 * CHUNK : (s0 + 2) * CHUNK], gps[:]
                        )
                        load["dve"] += _dve_cost(2 * CHUNK)

                def emit_v_group(g):
                    vps = ps_s.tile([C, 8, 32], f32, tag="squad", name=f"vps{g}")
                    for u in range(8):
                        t = 8 * g + u
                        nc.tensor.matmul(
                            vps[:, u, :], ones128row, bv32,
                            start=True, stop=False, skip_group_check=True,
                        )
                        nc.tensor.matmul(
                            vps[:, u, :], x_sb[:, t * 128 : (t + 1) * 128], wvb,
                            start=False, stop=True, skip_group_check=True,
                        )
                    nc.scalar.activation(
                        out=wexp[:, 8 * g : 8 * g + 8], in_=vps[:, :, 17], func=Exp
                    )
                    load["act"] += _act_cost(8)
                    nc.vector.tensor_tensor(
                        out=v_sb[:, 8 * g : 8 * g + 8, :],
                        in0=vps[:],
                        in1=wexp[:, 8 * g : 8 * g + 8].broadcast_to([C, 8, 32]),
                        op=mybir.AluOpType.mult,
                    )
                    load["dve"] += _dve_cost(256)

                emit_g_slice(0, split=True)
                emit_v_group(0)

                # ---- main loop: flat global schedule, cross-boundary AV lag ----
                NG = 16  # groups (== DR pairs) per chunk

                def emit_sts(c, gi, squad):
                    csl = slice(c * CHUNK, (c + 1) * CHUNK)
                    for t in range(2):
                        j = 2 * gi + t
                        nc.tensor.matmul(
                            squad[:, t, :],
                            x_sb[:, j * 128 : (j + 1) * 128],
                            g_sb[:, csl],
                            start=True, stop=True,
                        )

                def emit_exp(gi, squad, parity=0):
                    j0 = 2 * gi
                    if abs(load["act"] - load["dve"]) > 1400.0:
                        eng = pick_engine()
                    else:
                        eng = "act" if parity == 0 else "dve"
                    if eng == "act":
                        nc.scalar.activation(
                            out=et[:, j0 : j0 + 2, :],
                            in_=squad[:],
                            func=Exp,
                            bias=negshift,
                        )
                        load["act"] += _act_cost(2 * CHUNK)
                    else:
                        nc.vector.tensor_scalar(
                            out=et[:, j0 : j0 + 2, :].bitcast(u8),
                            in0=squad[:],
                            scalar1=A8,
                            scalar2=B8P,
                            op0=mybir.AluOpType.mult,
                            op1=mybir.AluOpType.add,
                        )
                        load["dve"] += _dve_cost(2 * CHUNK)

                def emit_av(acc_c, p):
                    nc.tensor.matmul(
                        acc_c[:],
                        v_sb[:, 2 * p : 2 * p + 2, 0:17],
                        et[:, 2 * p : 2 * p + 2, :],
                        start=(p == 0),
                        stop=(p == NG - 1),
                        perf_mode=DR,
                        skip_group_check=True,
                    )

                def emit_epi_evac(acc_c):
                    acc_sb = accsbp.tile([17, CHUNK], f32, tag="accsb")
                    nc.scalar.copy(acc_sb[:], acc_c[:])
                    load["act"] += _act_cost(CHUNK)
                    return acc_sb

                def emit_epi_norm(acc_sb):
                    tps4 = ps_s.tile([C, 4, 18], f32, tag="squad")
                    for s in range(4):
                        nc.tensor.matmul(
                            tps4[:, s, :],
                            acc_sb[:, s * 128 : (s + 1) * 128],
                            idpad,
                            start=True, stop=True,
                        )
                    recip4 = episb.tile([C, 4], f32, tag="recip")
                    nc.vector.reciprocal(recip4[:], tps4[:, :, 16])
                    load["dve"] += _dve_cost(4)
                    onorm4 = episb.tile([C, 4, 16], bf16, tag="onorm")
                    nc.vector.tensor_tensor(
                        out=onorm4[:],
                        in0=tps4[:, :, 0:16],
                        in1=recip4[:].broadcast_to([C, 4, 16]),
                        op=mybir.AluOpType.mult,
                    )
                    load["dve"] += _dve_cost(64)
                    return onorm4

                def emit_epi_proj(onorm4, c_prev):
                    pps = ps_epi.tile([D, CHUNK], f32, tag="pps")
                    nc.tensor.matmul(
                        pps[:], ones16, bp512,
                        start=True, stop=False, skip_group_check=True,
                    )
                    for s in range(4):
                        nc.tensor.matmul(
                            pps[:, s * 128 : (s + 1) * 128],
                            onorm4[:, s, :],
                            wpbf,
                            start=False, stop=(s == 3), skip_group_check=True,
                        )
                    osb = episb.tile([D, CHUNK], f32, tag="osb")
                    nc.scalar.copy(osb[:], pps[:])
                    load["act"] += _act_cost(CHUNK)
                    nc.sync.dma_start(
                        out=out_d[:, c_prev * CHUNK : (c_prev + 1) * CHUNK],
                        in_=osb[:],
                    )

                AV_LAG = 4
                _pro = {
                    1: lambda: emit_g_slice(2),
                    3: lambda: emit_v_group(1),
                    5: lambda: emit_g_slice(4),
                    7: lambda: emit_v_group(2),
                    9: lambda: emit_g_slice(6),
                    11: lambda: emit_v_group(3),
                }
                accs = {}
                epi = {}  # c -> dict of staged products
                av_next = 0  # global AV emission cursor (over 128 pairs)
                for G in range(NCHUNK * NG):
                    c, gi = divmod(G, NG)
                    squad = ps_s.tile([C, 2, CHUNK], f32, tag="squad")
                    emit_sts(c, gi, squad)
                    emit_exp(gi, squad, parity=G % 2)
                    if c == 0 and gi in _pro:
                        _pro.pop(gi)()
                    # drain AV pairs whose exp is AV_LAG groups back
                    while av_next <= G - AV_LAG:
                        cp, p = divmod(av_next, NG)
                        if p == 0:
                            accs[cp] = ps_acc.tile([17, CHUNK], f32, tag="acc", name=f"acc{cp}")
                        emit_av(accs[cp], p)
                        av_next += 1
                    if gi == 4 and c > 0:
                        epi[c - 1] = {"acc_sb": emit_epi_evac(accs.pop(c - 1))}
                    if gi == 5 and c > 0:
                        epi[c - 1]["onorm"] = emit_epi_norm(epi[c - 1]["acc_sb"])
                    if gi == 9 and c > 0:
                        emit_epi_proj(epi.pop(c - 1)["onorm"], c - 1)
                while av_next < NCHUNK * NG:
                    cp, p = divmod(av_next, NG)
                    if p == 0:
                        accs[cp] = ps_acc.tile([17, CHUNK], f32, tag="acc", name=f"acc{cp}")
                    emit_av(accs[cp], p)
                    av_next += 1
                c_last = NCHUNK - 1
                acc_sb = emit_epi_evac(accs.pop(c_last))
                emit_epi_proj(emit_epi_norm(acc_sb), c_last)

    nc.compile()
    return nc


def _get_program():
    if "nc" not in _CACHE:
        _CACHE["nc"] = _build()
    return _CACHE["nc"]


def _make_in_maps(x, w_qkv, b_qkv, w_proj, b_proj):
    import ml_dtypes

    bf = ml_dtypes.bfloat16
    x_cl = np.ascontiguousarray(
        np.asarray(x, dtype=np.float32).reshape(C, L).astype(bf)
    )
    w_qkv = np.asarray(w_qkv, dtype=np.float32)
    b_qkv = np.asarray(b_qkv, dtype=np.float32)
    w_proj = np.asarray(w_proj, dtype=np.float32)
    b_proj = np.asarray(b_proj, dtype=np.float32)
    wpT = np.ascontiguousarray(w_proj.T)

    cbase = np.zeros((C, CB_W), dtype=np.float32)
    cbase[0:17, 0:17] = np.eye(17, dtype=np.float32)  # idpad (col 17 zero)
    cbase[:, 18] = -SHIFT

    in_maps = []
    for i in range(N_CORES):
        rows_q = np.arange(D) * 24 + i * 3
        Wq = w_qkv[rows_q]
        Wk = w_qkv[rows_q + 1]
        Wv = w_qkv[rows_q + 2]
        bq = b_qkv[rows_q]
        bv = b_qkv[rows_q + 2]

        bbl = np.zeros((C, BB_W), dtype=bf)
        bbl[:, 0:128] = wpT.astype(bf)
        wvb = np.zeros((C, 32), dtype=np.float32)
        wvb[:, 0:16] = Wv.T
        wvb[:, 17] = 4.0 * (Wk.T @ bq)
        bbl[:, 128:160] = wvb.astype(bf)
        bbl[:, 160:288] = (4.0 * (Wq.T @ Wk)).astype(bf)
        bbl[0, 288:800] = np.tile(b_proj, 4).astype(bf)
        bbl[0, 800:816] = np.ones(16, dtype=bf)
        bv32 = np.zeros(32, dtype=np.float32)
        bv32[0:16] = bv
        bv32[16] = 1.0
        bbl[0, 816:848] = bv32.astype(bf)
        bbl[0, 848:976] = np.ones(128, dtype=bf)
        in_maps.append({"x_cl": x_cl, "cblob": cbase, "bblob": bbl})
    return in_maps


def _run(in_maps, trace=False):
    from concourse.bass_utils import run_bass_kernel_spmd

    nc = _get_program()
    return run_bass_kernel_spmd(nc, in_maps, list(range(N_CORES)), trace=trace)


def _assemble(results):
    out = np.empty((1, C, H, W), dtype=np.float32)
    for i in range(N_CORES):
        out[0, i * D : (i + 1) * D] = results[i]["out"].reshape(D, H, W)
    return out


def kernel(x, w_qkv, b_qkv, w_proj, b_proj):
    in_maps = _make_in_maps(x, w_qkv, b_qkv, w_proj, b_proj)
    r = _run(in_maps, trace=False)
    return _assemble(r.results)


def kernel_with_timing(x, w_qkv, b_qkv, w_proj, b_proj):
    """Like kernel() but also returns an HW execution time estimate in ns."""
    in_maps = _make_in_maps(x, w_qkv, b_qkv, w_proj, b_proj)
    try:
        r = _run(in_maps, trace=True)
        exec_ns = r.exec_time_ns
    except ModuleNotFoundError:
        r = _run(in_maps, trace=False)
        exec_ns = None
    if exec_ns is None:
        exec_ns = _CACHE.get("tlsim_ns")
        if exec_ns is None:
            from concourse.timeline_sim import TimelineSim

            exec_ns = int(TimelineSim(_get_program()).simulate())
            _CACHE["tlsim_ns"] = exec_ns
    return _assemble(r.results), exec_ns


# revision 20
# speedup vs baseline: 1.0025x; 1.0025x over previous
"""Trainium2 Bass kernel for nn_AttnBlock (B=1, C=128, H=32, W=128, 8 heads).

Sharding: one attention head per NeuronCore (8 heads / 8 cores). Each core
computes its head's attention over L=4096 positions and the final W-axis
projection for its 16-channel output slab. Host gathers 8 slabs.

v2 design (vs the flash baseline):
  * S^T via the rank-16 factor-through-weights trick: G = (4*Wk^T Wq) @ x is
    computed once on-device (128-deep contractions), then every S^T tile is
    x_tile^T @ G_chunk -- no q/k tensors, one evacuation (G) instead of two.
  * q-bias folded EXACTLY into a per-key reweighting of V: softmax(q_i.k_j +
    bq.k_j + const_i) => multiply [v_j|1] by w_j = exp(4*bq.k_j). The bias
    row 4*bq.k_j is produced as an extra column of the v matmul and exp'd on
    ScalarE; k-bias and bq.bk terms cancel in softmax exactly.
  * exp tiles (128 x 1536) split between ScalarE (exact exp -> fp8e4) and
    VectorE (Schraudolph int-bit trick -> uint8 saturating -> fp8e4 bits;
    negative-bit underflow saturates to 0.0 which is the correct flush).
  * A@V in fp8e4 with perf_mode=DoubleRow: one matmul contracts TWO j-tiles
    ([128, 2, 17] weights x [128, 2, 512] moving), with the softmax
    denominator as a 17th weight column (ones*w_j).
  * epilogue: transpose via idmatmul, batched reciprocal + broadcast-multiply
    normalize, W-axis projection in bf16.
"""

import math as _math

import numpy as np

N_CORES = 8
C = 128
H = 32
W = 128
L = H * W  # 4096
F = 8  # heads
D = 16  # head dim
CHUNK = 512
NCHUNK = L // CHUNK  # 8
NJT = L // 128  # 32 j-tiles
SHIFT = 2.5  # global exp shift for fp8 range (cancels in softmax)
A8 = 8.0 / _math.log(2.0)  # Schraudolph scale for e4m3 bits
B8P = (56.0 - 0.5) - SHIFT * A8  # magic + shift folded
CB_W = 20  # f32 cblob: idpad (17,18) | negshift col
# bf16 blob: wpbf 0:128 | wvb 128:160 | gw 160:288 | row0: bp512 288:800,
# ones16 800:816, bv32 816:848, ones128 848:976
BB_W = 976

# cost-model constants for build-time ACT/DVE load balancing (ns)
_ACT_CY = 1e9 / 1.2e9
_DVE_CY = 1e9 / 0.96e9


def _act_cost(fd):
    return (fd + 222) * _ACT_CY


def _dve_cost(fd, psum=True):
    return (fd + (120 if psum else 58)) * _DVE_CY


_CACHE = {}


def _build():
    import concourse.tile as tile
    from concourse import bacc, mybir

    f32 = mybir.dt.float32
    bf16 = mybir.dt.bfloat16
    fp8 = mybir.dt.float8e4
    u8 = mybir.dt.uint8
    Exp = mybir.ActivationFunctionType.Exp
    DR = mybir.MatmulPerfMode.DoubleRow

    nc = bacc.Bacc("TRN2", target_bir_lowering=False, debug=False)

    x_d = nc.dram_tensor("x_cl", [C, L], bf16, kind="ExternalInput").ap()
    cb_d = nc.dram_tensor("cblob", [C, CB_W], f32, kind="ExternalInput").ap()
    bb_d = nc.dram_tensor("bblob", [C, BB_W], bf16, kind="ExternalInput").ap()
    out_d = nc.dram_tensor("out", [D, L], f32, kind="ExternalOutput").ap()

    # build-time engine load (ns) for balancing flexible work
    load = {"act": 1283.0, "dve": 0.0}  # act table load charged up front

    def pick_engine():
        return "act" if load["act"] <= load["dve"] else "dve"

    with tile.TileContext(nc) as tc:
        with (
            tc.tile_pool(name="consts", bufs=1) as consts,
            tc.tile_pool(name="accsb", bufs=2) as accsbp,
            tc.tile_pool(name="episb", bufs=4) as episb,
        ):
            cb = consts.tile([C, CB_W], f32)
            idpad = cb[0:17, 0:18]
            negshift = cb[:, 18:19]
            bb = consts.tile([C, BB_W], bf16)
            wpbf = bb[:, 0:128]
            wvb = bb[:, 128:160]
            gw = bb[:, 160:288]
            bp512 = bb[0:1, 288:800]
            ones16 = bb[0:1, 800:816]
            bv32 = bb[0:1, 816:848]
            ones128row = bb[0:1, 848:976]

            x_sb = consts.tile([C, L], bf16)

            def dma_x(cch, q):
                q.dma_start(
                    out=x_sb[:, cch * CHUNK : (cch + 1) * CHUNK],
                    in_=x_d[:, cch * CHUNK : (cch + 1) * CHUNK],
                )

            dma_x(0, nc.sync)
            dma_x(1, nc.gpsimd)
            nc.sync.dma_start(out=bb, in_=bb_d)
            nc.scalar.dma_start(out=cb, in_=cb_d)
            dma_x(2, nc.scalar)
            dma_x(3, nc.gpsimd)
            dma_x(4, nc.sync)
            dma_x(5, nc.gpsimd)
            dma_x(6, nc.scalar)
            dma_x(7, nc.gpsimd)

            g_sb = consts.tile([C, L], bf16)
            et = consts.tile([C, NJT, CHUNK], fp8)
            v_sb = consts.tile([C, NJT, 32], fp8)
            wexp = consts.tile([C, NJT], f32)

            with (
                tc.tile_pool(name="ps_s", bufs=3, space="PSUM") as ps_s,
                tc.tile_pool(name="ps_acc", bufs=1, space="PSUM") as ps_acc,
                tc.tile_pool(name="ps_epi", bufs=1, space="PSUM") as ps_epi,
            ):
                # warm the ACT exp table immediately (no DMA dependency)
                dummy = episb.tile([1, 2], f32, tag="dummy")
                nc.gpsimd.memset(dummy[:], 0.5)
                nc.scalar.activation(out=dummy[:], in_=dummy[:], func=Exp)

                # ---- G = (4 Wk^T Wq) @ x and v-tile helpers; slice 0
                # upfront, the rest interleaved into chunk 0's schedule ----
                def emit_g_slice(s0, split=False):
                    gps = ps_s.tile([C, 2, CHUNK], f32, tag="squad", name=f"gps{s0}")
                    for t in range(2):
                        sl = slice((s0 + t) * CHUNK, (s0 + t + 1) * CHUNK)
                        nc.tensor.matmul(
                            gps[:, t, :], gw, x_sb[:, sl], start=True, stop=True
                        )
                        if split:
                            nc.vector.tensor_copy(
                                g_sb[:, (s0 + t) * CHUNK : (s0 + t + 1) * CHUNK],
                                gps[:, t, :],
                            )
                            load["dve"] += _dve_cost(CHUNK)
                    if not split:
                        nc.vector.tensor_copy(
                            g_sb[:, s0 * CHUNK : (s0 + 2) * CHUNK], gps[:]
                        )
                        load["dve"] += _dve_cost(2 * CHUNK)

                def emit_v_group(g):
                    vps = ps_s.tile([C, 8, 32], f32, tag="squad", name=f"vps{g}")
                    for u in range(8):
                        t = 8 * g + u
                        nc.tensor.matmul(
                            vps[:, u, :], ones128row, bv32,
                            start=True, stop=False, skip_group_check=True,
                        )
                        nc.tensor.matmul(
                            vps[:, u, :], x_sb[:, t * 128 : (t + 1) * 128], wvb,
                            start=False, stop=True, skip_group_check=True,
                        )
                    nc.scalar.activation(
                        out=wexp[:, 8 * g : 8 * g + 8], in_=vps[:, :, 17], func=Exp
                    )
                    load["act"] += _act_cost(8)
                    nc.vector.tensor_tensor(
                        out=v_sb[:, 8 * g : 8 * g + 8, :],
                        in0=vps[:],
                        in1=wexp[:, 8 * g : 8 * g + 8].broadcast_to([C, 8, 32]),
                        op=mybir.AluOpType.mult,
                    )
                    load["dve"] += _dve_cost(256)

                emit_g_slice(0, split=True)
                emit_v_group(0)

                # ---- main loop: flat global schedule, cross-boundary AV lag ----
                NG = 16  # groups (== DR pairs) per chunk

                def emit_sts(c, gi, squad):
                    csl = slice(c * CHUNK, (c + 1) * CHUNK)
                    for t in range(2):
                        j = 2 * gi + t
                        nc.tensor.matmul(
                            squad[:, t, :],
                            x_sb[:, j * 128 : (j + 1) * 128],
                            g_sb[:, csl],
                            start=True, stop=True,
                        )

                def emit_exp(gi, squad, parity=0):
                    j0 = 2 * gi
                    if abs(load["act"] - load["dve"]) > 999999.0:
                        eng = pick_engine()
                    else:
                        eng = "act" if parity == 0 else "dve"
                    if eng == "act":
                        nc.scalar.activation(
                            out=et[:, j0 : j0 + 2, :],
                            in_=squad[:],
                            func=Exp,
                            bias=negshift,
                        )
                        load["act"] += _act_cost(2 * CHUNK)
                    else:
                        nc.vector.tensor_scalar(
                            out=et[:, j0 : j0 + 2, :].bitcast(u8),
                            in0=squad[:],
                            scalar1=A8,
                            scalar2=B8P,
                            op0=mybir.AluOpType.mult,
                            op1=mybir.AluOpType.add,
                        )
                        load["dve"] += _dve_cost(2 * CHUNK)

                def emit_av(acc_c, p):
                    nc.tensor.matmul(
                        acc_c[:],
                        v_sb[:, 2 * p : 2 * p + 2, 0:17],
                        et[:, 2 * p : 2 * p + 2, :],
                        start=(p == 0),
                        stop=(p == NG - 1),
                        perf_mode=DR,
                        skip_group_check=True,
                    )

                def emit_epi_evac(acc_c):
                    acc_sb = accsbp.tile([17, CHUNK], f32, tag="accsb")
                    nc.scalar.copy(acc_sb[:], acc_c[:])
                    load["act"] += _act_cost(CHUNK)
                    return acc_sb

                def emit_epi_norm(acc_sb):
                    tps4 = ps_s.tile([C, 4, 18], f32, tag="squad")
                    for s in range(4):
                        nc.tensor.matmul(
                            tps4[:, s, :],
                            acc_sb[:, s * 128 : (s + 1) * 128],
                            idpad,
                            start=True, stop=True,
                        )
                    recip4 = episb.tile([C, 4], f32, tag="recip")
                    nc.vector.reciprocal(recip4[:], tps4[:, :, 16])
                    load["dve"] += _dve_cost(4)
                    onorm4 = episb.tile([C, 4, 16], bf16, tag="onorm")
                    nc.vector.tensor_tensor(
                        out=onorm4[:],
                        in0=tps4[:, :, 0:16],
                        in1=recip4[:].broadcast_to([C, 4, 16]),
                        op=mybir.AluOpType.mult,
                    )
                    load["dve"] += _dve_cost(64)
                    return onorm4

                def emit_epi_proj(onorm4, c_prev):
                    pps = ps_epi.tile([D, CHUNK], f32, tag="pps")
                    nc.tensor.matmul(
                        pps[:], ones16, bp512,
                        start=True, stop=False, skip_group_check=True,
                    )
                    for s in range(4):
                        nc.tensor.matmul(
                            pps[:, s * 128 : (s + 1) * 128],
                            onorm4[:, s, :],
                            wpbf,
                            start=False, stop=(s == 3), skip_group_check=True,
                        )
                    osb = episb.tile([D, CHUNK], f32, tag="osb")
                    nc.scalar.copy(osb[:], pps[:])
                    load["act"] += _act_cost(CHUNK)
                    nc.sync.dma_start(
                        out=out_d[:, c_prev * CHUNK : (c_prev + 1) * CHUNK],
                        in_=osb[:],
                    )

                AV_LAG = 4
                _pro = {
                    1: lambda: emit_g_slice(2),
                    3: lambda: emit_v_group(1),
                    5: lambda: emit_g_slice(4),
                    7: lambda: emit_v_group(2),
                    9: lambda: emit_g_slice(6),
                    11: lambda: emit_v_group(3),
                }
                accs = {}
                epi = {}  # c -> dict of staged products
                av_next = 0  # global AV emission cursor (over 128 pairs)
                for G in range(NCHUNK * NG):
                    c, gi = divmod(G, NG)
                    squad = ps_s.tile([C, 2, CHUNK], f32, tag="squad")
                    emit_sts(c, gi, squad)
                    emit_exp(gi, squad, parity=G % 2)
                    if c == 0 and gi in _pro:
                        _pro.pop(gi)()
                    # drain AV pairs whose exp is AV_LAG groups back
                    while av_next <= G - AV_LAG:
                        cp, p = divmod(av_next, NG)
                        if p == 0:
                            accs[cp] = ps_acc.tile([17, CHUNK], f32, tag="acc", name=f"acc{cp}")
                        emit_av(accs[cp], p)
                        av_next += 1
                    if gi == 4 and c > 0:
                        epi[c - 1] = {"acc_sb": emit_epi_evac(accs.pop(c - 1))}
                    if gi == 5 and c > 0:
                        epi[c - 1]["onorm"] = emit_epi_norm(epi[c - 1]["acc_sb"])
                    if gi == 9 and c > 0:
                        emit_epi_proj(epi.pop(c - 1)["onorm"], c - 1)
                while av_next < NCHUNK * NG:
                    cp, p = divmod(av_next, NG)
                    if p == 0:
                        accs[cp] = ps_acc.tile([17, CHUNK], f32, tag="acc", name=f"acc{cp}")
                    emit_av(accs[cp], p)
                    av_next += 1
                c_last = NCHUNK - 1
                acc_sb = emit_epi_evac(accs.pop(c_last))
                emit_epi_proj(emit_epi_norm(acc_sb), c_last)

    nc.compile()
    return nc


def _get_program():
    if "nc" not in _CACHE:
        _CACHE["nc"] = _build()
    return _CACHE["nc"]


def _make_in_maps(x, w_qkv, b_qkv, w_proj, b_proj):
    import ml_dtypes

    bf = ml_dtypes.bfloat16
    x_cl = np.ascontiguousarray(
        np.asarray(x, dtype=np.float32).reshape(C, L).astype(bf)
    )
    w_qkv = np.asarray(w_qkv, dtype=np.float32)
    b_qkv = np.asarray(b_qkv, dtype=np.float32)
    w_proj = np.asarray(w_proj, dtype=np.float32)
    b_proj = np.asarray(b_proj, dtype=np.float32)
    wpT = np.ascontiguousarray(w_proj.T)

    cbase = np.zeros((C, CB_W), dtype=np.float32)
    cbase[0:17, 0:17] = np.eye(17, dtype=np.float32)  # idpad (col 17 zero)
    cbase[:, 18] = -SHIFT

    in_maps = []
    for i in range(N_CORES):
        rows_q = np.arange(D) * 24 + i * 3
        Wq = w_qkv[rows_q]
        Wk = w_qkv[rows_q + 1]
        Wv = w_qkv[rows_q + 2]
        bq = b_qkv[rows_q]
        bv = b_qkv[rows_q + 2]

        bbl = np.zeros((C, BB_W), dtype=bf)
        bbl[:, 0:128] = wpT.astype(bf)
        wvb = np.zeros((C, 32), dtype=np.float32)
        wvb[:, 0:16] = Wv.T
        wvb[:, 17] = 4.0 * (Wk.T @ bq)
        bbl[:, 128:160] = wvb.astype(bf)
        bbl[:, 160:288] = (4.0 * (Wq.T @ Wk)).astype(bf)
        bbl[0, 288:800] = np.tile(b_proj, 4).astype(bf)
        bbl[0, 800:816] = np.ones(16, dtype=bf)
        bv32 = np.zeros(32, dtype=np.float32)
        bv32[0:16] = bv
        bv32[16] = 1.0
        bbl[0, 816:848] = bv32.astype(bf)
        bbl[0, 848:976] = np.ones(128, dtype=bf)
        in_maps.append({"x_cl": x_cl, "cblob": cbase, "bblob": bbl})
    return in_maps


def _run(in_maps, trace=False):
    from concourse.bass_utils import run_bass_kernel_spmd

    nc = _get_program()
    return run_bass_kernel_spmd(nc, in_maps, list(range(N_CORES)), trace=trace)


def _assemble(results):
    out = np.empty((1, C, H, W), dtype=np.float32)
    for i in range(N_CORES):
        out[0, i * D : (i + 1) * D] = results[i]["out"].reshape(D, H, W)
    return out


def kernel(x, w_qkv, b_qkv, w_proj, b_proj):
    in_maps = _make_in_maps(x, w_qkv, b_qkv, w_proj, b_proj)
    r = _run(in_maps, trace=False)
    return _assemble(r.results)


def kernel_with_timing(x, w_qkv, b_qkv, w_proj, b_proj):
    """Like kernel() but also returns an HW execution time estimate in ns."""
    in_maps = _make_in_maps(x, w_qkv, b_qkv, w_proj, b_proj)
    try:
        r = _run(in_maps, trace=True)
        exec_ns = r.exec_time_ns
    except ModuleNotFoundError:
        r = _run(in_maps, trace=False)
        exec_ns = None
    if exec_ns is None:
        exec_ns = _CACHE.get("tlsim_ns")
        if exec_ns is None:
            from concourse.timeline_sim import TimelineSim

            exec_ns = int(TimelineSim(_get_program()).simulate())
            _CACHE["tlsim_ns"] = exec_ns
    return _assemble(r.results), exec_ns


# revision 21
# speedup vs baseline: 1.0201x; 1.0175x over previous
"""Trainium2 Bass kernel for nn_AttnBlock (B=1, C=128, H=32, W=128, 8 heads).

Sharding: one attention head per NeuronCore (8 heads / 8 cores). Each core
computes its head's attention over L=4096 positions and the final W-axis
projection for its 16-channel output slab. Host gathers 8 slabs.

v2 design (vs the flash baseline):
  * S^T via the rank-16 factor-through-weights trick: G = (4*Wk^T Wq) @ x is
    computed once on-device (128-deep contractions), then every S^T tile is
    x_tile^T @ G_chunk -- no q/k tensors, one evacuation (G) instead of two.
  * q-bias folded EXACTLY into a per-key reweighting of V: softmax(q_i.k_j +
    bq.k_j + const_i) => multiply [v_j|1] by w_j = exp(4*bq.k_j). The bias
    row 4*bq.k_j is produced as an extra column of the v matmul and exp'd on
    ScalarE; k-bias and bq.bk terms cancel in softmax exactly.
  * exp tiles (128 x 1536) split between ScalarE (exact exp -> fp8e4) and
    VectorE (Schraudolph int-bit trick -> uint8 saturating -> fp8e4 bits;
    negative-bit underflow saturates to 0.0 which is the correct flush).
  * A@V in fp8e4 with perf_mode=DoubleRow: one matmul contracts TWO j-tiles
    ([128, 2, 17] weights x [128, 2, 512] moving), with the softmax
    denominator as a 17th weight column (ones*w_j).
  * epilogue: transpose via idmatmul, batched reciprocal + broadcast-multiply
    normalize, W-axis projection in bf16.
"""

import math as _math

import numpy as np

N_CORES = 8
C = 128
H = 32
W = 128
L = H * W  # 4096
F = 8  # heads
D = 16  # head dim
CHUNK = 512
NCHUNK = L // CHUNK  # 8
NJT = L // 128  # 32 j-tiles
SHIFT = 2.5  # global exp shift for fp8 range (cancels in softmax)
A8 = 8.0 / _math.log(2.0)  # Schraudolph scale for e4m3 bits
B8P = (56.0 - 0.5) - SHIFT * A8  # magic + shift folded
CB_W = 20  # f32 cblob: idpad (17,18) | negshift col
# bf16 blob: wpbf 0:128 | wvb 128:160 | gw 160:288 | row0: bp512 288:800,
# ones16 800:816, bv32 816:848, ones128 848:976
BB_W = 976

# cost-model constants for build-time ACT/DVE load balancing (ns)
_ACT_CY = 1e9 / 1.2e9
_DVE_CY = 1e9 / 0.96e9


def _act_cost(fd):
    return (fd + 222) * _ACT_CY


def _dve_cost(fd, psum=True):
    return (fd + (120 if psum else 58)) * _DVE_CY


_CACHE = {}


def _build():
    import concourse.tile as tile
    from concourse import bacc, mybir

    f32 = mybir.dt.float32
    bf16 = mybir.dt.bfloat16
    fp8 = mybir.dt.float8e4
    u8 = mybir.dt.uint8
    Exp = mybir.ActivationFunctionType.Exp
    DR = mybir.MatmulPerfMode.DoubleRow

    nc = bacc.Bacc("TRN2", target_bir_lowering=False, debug=False)

    x_d = nc.dram_tensor("x_cl", [C, L], bf16, kind="ExternalInput").ap()
    x8_d = nc.dram_tensor("x8", [C // 2, 2, L], fp8, kind="ExternalInput").ap()
    cb_d = nc.dram_tensor("cblob", [C, CB_W], f32, kind="ExternalInput").ap()
    bb_d = nc.dram_tensor("bblob", [C, BB_W], bf16, kind="ExternalInput").ap()
    out_d = nc.dram_tensor("out", [D, L], f32, kind="ExternalOutput").ap()

    # build-time engine load (ns) for balancing flexible work
    load = {"act": 1283.0, "dve": 0.0}  # act table load charged up front

    def pick_engine():
        return "act" if load["act"] <= load["dve"] else "dve"

    with tile.TileContext(nc) as tc:
        with (
            tc.tile_pool(name="consts", bufs=1) as consts,
            tc.tile_pool(name="accsb", bufs=2) as accsbp,
            tc.tile_pool(name="episb", bufs=4) as episb,
        ):
            cb = consts.tile([C, CB_W], f32)
            idpad = cb[0:17, 0:18]
            negshift = cb[:, 18:19]
            bb = consts.tile([C, BB_W], bf16)
            wpbf = bb[:, 0:128]
            wvb = bb[:, 128:160]
            gw = bb[:, 160:288]
            bp512 = bb[0:1, 288:800]
            ones16 = bb[0:1, 800:816]
            bv32 = bb[0:1, 816:848]
            ones128row = bb[0:1, 848:976]

            x_sb = consts.tile([C, L], bf16)

            def dma_x(cch, q):
                q.dma_start(
                    out=x_sb[:, cch * CHUNK : (cch + 1) * CHUNK],
                    in_=x_d[:, cch * CHUNK : (cch + 1) * CHUNK],
                )

            dma_x(0, nc.sync)
            dma_x(1, nc.gpsimd)
            nc.sync.dma_start(out=bb, in_=bb_d)
            nc.scalar.dma_start(out=cb, in_=cb_d)
            dma_x(2, nc.scalar)
            dma_x(3, nc.gpsimd)
            dma_x(4, nc.sync)
            dma_x(5, nc.gpsimd)
            dma_x(6, nc.scalar)
            dma_x(7, nc.gpsimd)

            x8_sb = consts.tile([C // 2, 2, L], fp8)
            nc.scalar.dma_start(out=x8_sb, in_=x8_d)
            g_f8 = consts.tile([C, L], fp8)
            g8_sb = consts.tile([C // 2, 2, L], fp8)
            et = consts.tile([C, NJT, CHUNK], fp8)
            v_sb = consts.tile([C, NJT, 32], fp8)
            wexp = consts.tile([C, NJT], f32)

            with (
                tc.tile_pool(name="ps_s", bufs=3, space="PSUM") as ps_s,
                tc.tile_pool(name="ps_acc", bufs=1, space="PSUM") as ps_acc,
                tc.tile_pool(name="ps_epi", bufs=1, space="PSUM") as ps_epi,
            ):
                # warm the ACT exp table immediately (no DMA dependency)
                dummy = episb.tile([1, 2], f32, tag="dummy")
                nc.gpsimd.memset(dummy[:], 0.5)
                nc.scalar.activation(out=dummy[:], in_=dummy[:], func=Exp)

                # ---- G = (4 Wk^T Wq) @ x and v-tile helpers; slice 0
                # upfront, the rest interleaved into chunk 0's schedule ----
                def emit_g_slice(s0, split=False):
                    gps = ps_s.tile([C, 2, CHUNK], f32, tag="squad", name=f"gps{s0}")
                    sl01 = slice(s0 * CHUNK, (s0 + 2) * CHUNK)
                    for t in range(2):
                        sl = slice((s0 + t) * CHUNK, (s0 + t + 1) * CHUNK)
                        nc.tensor.matmul(
                            gps[:, t, :], gw, x_sb[:, sl], start=True, stop=True
                        )
                        if split:
                            nc.scalar.copy(
                                g_f8[:, (s0 + t) * CHUNK : (s0 + t + 1) * CHUNK],
                                gps[:, t, :],
                            )
                            load["act"] += _act_cost(CHUNK)
                    if not split:
                        nc.scalar.copy(g_f8[:, sl01], gps[:])
                        load["act"] += _act_cost(2 * CHUNK)
                    # pair-parity partition permute via DMA (free track):
                    # g8[u, e, i] = g_f8[2u + e, i]
                    for e in range(2):
                        nc.sync.dma_start(
                            out=g8_sb[:, e, sl01], in_=g_f8[e : C : 2, sl01]
                        )

                def emit_v_group(g):
                    vps = ps_s.tile([C, 8, 32], f32, tag="squad", name=f"vps{g}")
                    for u in range(8):
                        t = 8 * g + u
                        nc.tensor.matmul(
                            vps[:, u, :], ones128row, bv32,
                            start=True, stop=False, skip_group_check=True,
                        )
                        nc.tensor.matmul(
                            vps[:, u, :], x_sb[:, t * 128 : (t + 1) * 128], wvb,
                            start=False, stop=True, skip_group_check=True,
                        )
                    nc.scalar.activation(
                        out=wexp[:, 8 * g : 8 * g + 8], in_=vps[:, :, 17], func=Exp
                    )
                    load["act"] += _act_cost(8)
                    nc.vector.tensor_tensor(
                        out=v_sb[:, 8 * g : 8 * g + 8, :],
                        in0=vps[:],
                        in1=wexp[:, 8 * g : 8 * g + 8].broadcast_to([C, 8, 32]),
                        op=mybir.AluOpType.mult,
                    )
                    load["dve"] += _dve_cost(256)

                emit_g_slice(0, split=True)
                emit_v_group(0)

                # ---- main loop: flat global schedule, cross-boundary AV lag ----
                NG = 16  # groups (== DR pairs) per chunk

                def emit_sts(c, gi, squad):
                    csl = slice(c * CHUNK, (c + 1) * CHUNK)
                    for t in range(2):
                        j = 2 * gi + t
                        nc.tensor.matmul(
                            squad[:, t, :],
                            x8_sb[:, :, j * 128 : (j + 1) * 128],
                            g8_sb[:, :, csl],
                            start=True, stop=True,
                            perf_mode=DR,
                        )

                def emit_exp(gi, squad, parity=0):
                    j0 = 2 * gi
                    if abs(load["act"] - load["dve"]) > 999999.0:
                        eng = pick_engine()
                    else:
                        eng = "act" if parity == 0 else "dve"
                    if eng == "act":
                        nc.scalar.activation(
                            out=et[:, j0 : j0 + 2, :],
                            in_=squad[:],
                            func=Exp,
                            bias=negshift,
                        )
                        load["act"] += _act_cost(2 * CHUNK)
                    else:
                        nc.vector.tensor_scalar(
                            out=et[:, j0 : j0 + 2, :].bitcast(u8),
                            in0=squad[:],
                            scalar1=A8,
                            scalar2=B8P,
                            op0=mybir.AluOpType.mult,
                            op1=mybir.AluOpType.add,
                        )
                        load["dve"] += _dve_cost(2 * CHUNK)

                def emit_av(acc_c, p):
                    nc.tensor.matmul(
                        acc_c[:],
                        v_sb[:, 2 * p : 2 * p + 2, 0:17],
                        et[:, 2 * p : 2 * p + 2, :],
                        start=(p == 0),
                        stop=(p == NG - 1),
                        perf_mode=DR,
                        skip_group_check=True,
                    )

                def emit_epi_evac(acc_c):
                    acc_sb = accsbp.tile([17, CHUNK], f32, tag="accsb")
                    nc.scalar.copy(acc_sb[:], acc_c[:])
                    load["act"] += _act_cost(CHUNK)
                    return acc_sb

                def emit_epi_norm(acc_sb):
                    tps4 = ps_s.tile([C, 4, 18], f32, tag="squad")
                    for s in range(4):
                        nc.tensor.matmul(
                            tps4[:, s, :],
                            acc_sb[:, s * 128 : (s + 1) * 128],
                            idpad,
                            start=True, stop=True,
                        )
                    recip4 = episb.tile([C, 4], f32, tag="recip")
                    nc.vector.reciprocal(recip4[:], tps4[:, :, 16])
                    load["dve"] += _dve_cost(4)
                    onorm4 = episb.tile([C, 4, 16], bf16, tag="onorm")
                    nc.vector.tensor_tensor(
                        out=onorm4[:],
                        in0=tps4[:, :, 0:16],
                        in1=recip4[:].broadcast_to([C, 4, 16]),
                        op=mybir.AluOpType.mult,
                    )
                    load["dve"] += _dve_cost(64)
                    return onorm4

                def emit_epi_proj(onorm4, c_prev):
                    pps = ps_epi.tile([D, CHUNK], f32, tag="pps")
                    nc.tensor.matmul(
                        pps[:], ones16, bp512,
                        start=True, stop=False, skip_group_check=True,
                    )
                    for s in range(4):
                        nc.tensor.matmul(
                            pps[:, s * 128 : (s + 1) * 128],
                            onorm4[:, s, :],
                            wpbf,
                            start=False, stop=(s == 3), skip_group_check=True,
                        )
                    osb = episb.tile([D, CHUNK], f32, tag="osb")
                    nc.scalar.copy(osb[:], pps[:])
                    load["act"] += _act_cost(CHUNK)
                    nc.sync.dma_start(
                        out=out_d[:, c_prev * CHUNK : (c_prev + 1) * CHUNK],
                        in_=osb[:],
                    )

                AV_LAG = 4
                _pro = {
                    1: lambda: emit_g_slice(2),
                    3: lambda: emit_v_group(1),
                    5: lambda: emit_g_slice(4),
                    7: lambda: emit_v_group(2),
                    9: lambda: emit_g_slice(6),
                    11: lambda: emit_v_group(3),
                }
                accs = {}
                epi = {}  # c -> dict of staged products
                av_next = 0  # global AV emission cursor (over 128 pairs)
                for G in range(NCHUNK * NG):
                    c, gi = divmod(G, NG)
                    squad = ps_s.tile([C, 2, CHUNK], f32, tag="squad")
                    emit_sts(c, gi, squad)
                    emit_exp(gi, squad, parity=G % 2)
                    if c == 0 and gi in _pro:
                        _pro.pop(gi)()
                    # drain AV pairs whose exp is AV_LAG groups back
                    while av_next <= G - AV_LAG:
                        cp, p = divmod(av_next, NG)
                        if p == 0:
                            accs[cp] = ps_acc.tile([17, CHUNK], f32, tag="acc", name=f"acc{cp}")
                        emit_av(accs[cp], p)
                        av_next += 1
                    if gi == 4 and c > 0:
                        epi[c - 1] = {"acc_sb": emit_epi_evac(accs.pop(c - 1))}
                    if gi == 5 and c > 0:
                        epi[c - 1]["onorm"] = emit_epi_norm(epi[c - 1]["acc_sb"])
                    if gi == 9 and c > 0:
                        emit_epi_proj(epi.pop(c - 1)["onorm"], c - 1)
                while av_next < NCHUNK * NG:
                    cp, p = divmod(av_next, NG)
                    if p == 0:
                        accs[cp] = ps_acc.tile([17, CHUNK], f32, tag="acc", name=f"acc{cp}")
                    emit_av(accs[cp], p)
                    av_next += 1
                c_last = NCHUNK - 1
                acc_sb = emit_epi_evac(accs.pop(c_last))
                emit_epi_proj(emit_epi_norm(acc_sb), c_last)

    nc.compile()
    return nc


def _get_program():
    if "nc" not in _CACHE:
        _CACHE["nc"] = _build()
    return _CACHE["nc"]


def _make_in_maps(x, w_qkv, b_qkv, w_proj, b_proj):
    import ml_dtypes

    bf = ml_dtypes.bfloat16
    x_f32 = np.asarray(x, dtype=np.float32).reshape(C, L)
    x_cl = np.ascontiguousarray(x_f32.astype(bf))
    x8 = np.ascontiguousarray(
        (x_f32 * 0.25).reshape(C // 2, 2, L).astype(ml_dtypes.float8_e4m3)
    )
    w_qkv = np.asarray(w_qkv, dtype=np.float32)
    b_qkv = np.asarray(b_qkv, dtype=np.float32)
    w_proj = np.asarray(w_proj, dtype=np.float32)
    b_proj = np.asarray(b_proj, dtype=np.float32)
    wpT = np.ascontiguousarray(w_proj.T)

    cbase = np.zeros((C, CB_W), dtype=np.float32)
    cbase[0:17, 0:17] = np.eye(17, dtype=np.float32)  # idpad (col 17 zero)
    cbase[:, 18] = -SHIFT

    in_maps = []
    for i in range(N_CORES):
        rows_q = np.arange(D) * 24 + i * 3
        Wq = w_qkv[rows_q]
        Wk = w_qkv[rows_q + 1]
        Wv = w_qkv[rows_q + 2]
        bq = b_qkv[rows_q]
        bv = b_qkv[rows_q + 2]

        bbl = np.zeros((C, BB_W), dtype=bf)
        bbl[:, 0:128] = wpT.astype(bf)
        wvb = np.zeros((C, 32), dtype=np.float32)
        wvb[:, 0:16] = Wv.T
        wvb[:, 17] = 4.0 * (Wk.T @ bq)
        bbl[:, 128:160] = wvb.astype(bf)
        bbl[:, 160:288] = (16.0 * (Wq.T @ Wk)).astype(bf)
        bbl[0, 288:800] = np.tile(b_proj, 4).astype(bf)
        bbl[0, 800:816] = np.ones(16, dtype=bf)
        bv32 = np.zeros(32, dtype=np.float32)
        bv32[0:16] = bv
        bv32[16] = 1.0
        bbl[0, 816:848] = bv32.astype(bf)
        bbl[0, 848:976] = np.ones(128, dtype=bf)
        in_maps.append({"x_cl": x_cl, "x8": x8, "cblob": cbase, "bblob": bbl})
    return in_maps


def _run(in_maps, trace=False):
    from concourse.bass_utils import run_bass_kernel_spmd

    nc = _get_program()
    return run_bass_kernel_spmd(nc, in_maps, list(range(N_CORES)), trace=trace)


def _assemble(results):
    out = np.empty((1, C, H, W), dtype=np.float32)
    for i in range(N_CORES):
        out[0, i * D : (i + 1) * D] = results[i]["out"].reshape(D, H, W)
    return out


def kernel(x, w_qkv, b_qkv, w_proj, b_proj):
    in_maps = _make_in_maps(x, w_qkv, b_qkv, w_proj, b_proj)
    r = _run(in_maps, trace=False)
    return _assemble(r.results)


def kernel_with_timing(x, w_qkv, b_qkv, w_proj, b_proj):
    """Like kernel() but also returns an HW execution time estimate in ns."""
    in_maps = _make_in_maps(x, w_qkv, b_qkv, w_proj, b_proj)
    try:
        r = _run(in_maps, trace=True)
        exec_ns = r.exec_time_ns
    except ModuleNotFoundError:
        r = _run(in_maps, trace=False)
        exec_ns = None
    if exec_ns is None:
        exec_ns = _CACHE.get("tlsim_ns")
        if exec_ns is None:
            from concourse.timeline_sim import TimelineSim

            exec_ns = int(TimelineSim(_get_program()).simulate())
            _CACHE["tlsim_ns"] = exec_ns
    return _assemble(r.results), exec_ns


# revision 22
# speedup vs baseline: 1.0247x; 1.0045x over previous
"""Trainium2 Bass kernel for nn_AttnBlock (B=1, C=128, H=32, W=128, 8 heads).

Sharding: one attention head per NeuronCore (8 heads / 8 cores). Each core
computes its head's attention over L=4096 positions and the final W-axis
projection for its 16-channel output slab. Host gathers 8 slabs.

v2 design (vs the flash baseline):
  * S^T via the rank-16 factor-through-weights trick: G = (4*Wk^T Wq) @ x is
    computed once on-device (128-deep contractions), then every S^T tile is
    x_tile^T @ G_chunk -- no q/k tensors, one evacuation (G) instead of two.
  * q-bias folded EXACTLY into a per-key reweighting of V: softmax(q_i.k_j +
    bq.k_j + const_i) => multiply [v_j|1] by w_j = exp(4*bq.k_j). The bias
    row 4*bq.k_j is produced as an extra column of the v matmul and exp'd on
    ScalarE; k-bias and bq.bk terms cancel in softmax exactly.
  * exp tiles (128 x 1536) split between ScalarE (exact exp -> fp8e4) and
    VectorE (Schraudolph int-bit trick -> uint8 saturating -> fp8e4 bits;
    negative-bit underflow saturates to 0.0 which is the correct flush).
  * A@V in fp8e4 with perf_mode=DoubleRow: one matmul contracts TWO j-tiles
    ([128, 2, 17] weights x [128, 2, 512] moving), with the softmax
    denominator as a 17th weight column (ones*w_j).
  * epilogue: transpose via idmatmul, batched reciprocal + broadcast-multiply
    normalize, W-axis projection in bf16.
"""

import math as _math

import numpy as np

N_CORES = 8
C = 128
H = 32
W = 128
L = H * W  # 4096
F = 8  # heads
D = 16  # head dim
CHUNK = 512
NCHUNK = L // CHUNK  # 8
NJT = L // 128  # 32 j-tiles
SHIFT = 2.5  # global exp shift for fp8 range (cancels in softmax)
A8 = 8.0 / _math.log(2.0)  # Schraudolph scale for e4m3 bits
B8P = (56.0 - 0.5) - SHIFT * A8  # magic + shift folded
CB_W = 20  # f32 cblob: idpad (17,18) | negshift col
# bf16 blob: wpbf 0:128 | wvb 128:160 | gw 160:288 | row0: bp512 288:800,
# ones16 800:816, bv32 816:848, ones128 848:976
BB_W = 976

# cost-model constants for build-time ACT/DVE load balancing (ns)
_ACT_CY = 1e9 / 1.2e9
_DVE_CY = 1e9 / 0.96e9


def _act_cost(fd):
    return (fd + 222) * _ACT_CY


def _dve_cost(fd, psum=True):
    return (fd + (120 if psum else 58)) * _DVE_CY


_CACHE = {}


def _build():
    import concourse.tile as tile
    from concourse import bacc, mybir

    f32 = mybir.dt.float32
    bf16 = mybir.dt.bfloat16
    fp8 = mybir.dt.float8e4
    u8 = mybir.dt.uint8
    Exp = mybir.ActivationFunctionType.Exp
    DR = mybir.MatmulPerfMode.DoubleRow

    nc = bacc.Bacc("TRN2", target_bir_lowering=False, debug=False)

    x_d = nc.dram_tensor("x_cl", [C, L], bf16, kind="ExternalInput").ap()
    x8_d = nc.dram_tensor("x8", [C // 2, 2, L], fp8, kind="ExternalInput").ap()
    cb_d = nc.dram_tensor("cblob", [C, CB_W], f32, kind="ExternalInput").ap()
    bb_d = nc.dram_tensor("bblob", [C, BB_W], bf16, kind="ExternalInput").ap()
    out_d = nc.dram_tensor("out", [D, L], f32, kind="ExternalOutput").ap()

    # build-time engine load (ns) for balancing flexible work
    load = {"act": 1283.0, "dve": 0.0}  # act table load charged up front

    def pick_engine():
        return "act" if load["act"] <= load["dve"] else "dve"

    with tile.TileContext(nc) as tc:
        with (
            tc.tile_pool(name="consts", bufs=1) as consts,
            tc.tile_pool(name="accsb", bufs=2) as accsbp,
            tc.tile_pool(name="episb", bufs=4) as episb,
        ):
            cb = consts.tile([C, CB_W], f32)
            idpad = cb[0:17, 0:18]
            negshift = cb[:, 18:19]
            bb = consts.tile([C, BB_W], bf16)
            wpbf = bb[:, 0:128]
            wvb = bb[:, 128:160]
            gw = bb[:, 160:288]
            bp512 = bb[0:1, 288:800]
            ones16 = bb[0:1, 800:816]
            bv32 = bb[0:1, 816:848]
            ones128row = bb[0:1, 848:976]

            x_sb = consts.tile([C, L], bf16)

            def dma_x(cch, q):
                q.dma_start(
                    out=x_sb[:, cch * CHUNK : (cch + 1) * CHUNK],
                    in_=x_d[:, cch * CHUNK : (cch + 1) * CHUNK],
                )

            x8_sb = consts.tile([C // 2, 2, L], fp8)
            g_f8 = consts.tile([C, L], fp8)
            g8_sb = consts.tile([C // 2, 2, L], fp8)

            # DMA orchestration: critical-path order.
            #   sync/scalar share the HWDGE; gpsimd uses SWDGE (Pool).
            nc.sync.dma_start(out=bb, in_=bb_d)
            dma_x(0, nc.sync)
            nc.scalar.dma_start(out=cb, in_=cb_d)
            dma_x(1, nc.gpsimd)
            for piece in range(4):
                nc.scalar.dma_start(
                    out=x8_sb[:, :, piece * 1024 : (piece + 1) * 1024],
                    in_=x8_d[:, :, piece * 1024 : (piece + 1) * 1024],
                )
            dma_x(2, nc.sync)
            dma_x(3, nc.gpsimd)
            dma_x(4, nc.sync)
            dma_x(5, nc.gpsimd)
            dma_x(6, nc.sync)
            dma_x(7, nc.gpsimd)
            et = consts.tile([C, NJT, CHUNK], fp8)
            v_sb = consts.tile([C, NJT, 32], fp8)
            wexp = consts.tile([C, NJT], f32)

            with (
                tc.tile_pool(name="ps_s", bufs=3, space="PSUM") as ps_s,
                tc.tile_pool(name="ps_acc", bufs=1, space="PSUM") as ps_acc,
                tc.tile_pool(name="ps_epi", bufs=1, space="PSUM") as ps_epi,
            ):
                # warm the ACT exp table immediately (no DMA dependency)
                dummy = episb.tile([1, 2], f32, tag="dummy")
                nc.gpsimd.memset(dummy[:], 0.5)
                nc.scalar.activation(out=dummy[:], in_=dummy[:], func=Exp)

                # ---- G = (4 Wk^T Wq) @ x and v-tile helpers; slice 0
                # upfront, the rest interleaved into chunk 0's schedule ----
                def emit_g_slice(s0, split=False):
                    # g8[u, e, i] = g_f8[2u + e, i] (partition-pair permute
                    # via DMA on the otherwise idle DMA track)
                    gps = ps_s.tile([C, 2, CHUNK], f32, tag="squad", name=f"gps{s0}")
                    sl01 = slice(s0 * CHUNK, (s0 + 2) * CHUNK)
                    for t in range(2):
                        sl = slice((s0 + t) * CHUNK, (s0 + t + 1) * CHUNK)
                        nc.tensor.matmul(
                            gps[:, t, :], gw, x_sb[:, sl], start=True, stop=True
                        )
                        if split:
                            nc.scalar.copy(g_f8[:, sl], gps[:, t, :])
                            load["act"] += _act_cost(CHUNK)
                            for e in range(2):
                                nc.sync.dma_start(
                                    out=g8_sb[:, e, sl], in_=g_f8[e : C : 2, sl]
                                )
                    if not split:
                        nc.scalar.copy(g_f8[:, sl01], gps[:])
                        load["act"] += _act_cost(2 * CHUNK)
                        for e in range(2):
                            nc.sync.dma_start(
                                out=g8_sb[:, e, sl01], in_=g_f8[e : C : 2, sl01]
                            )

                def emit_v_group(g):
                    vps = ps_s.tile([C, 8, 32], f32, tag="squad", name=f"vps{g}")
                    for u in range(8):
                        t = 8 * g + u
                        nc.tensor.matmul(
                            vps[:, u, :], ones128row, bv32,
                            start=True, stop=False, skip_group_check=True,
                        )
                        nc.tensor.matmul(
                            vps[:, u, :], x_sb[:, t * 128 : (t + 1) * 128], wvb,
                            start=False, stop=True, skip_group_check=True,
                        )
                    nc.scalar.activation(
                        out=wexp[:, 8 * g : 8 * g + 8], in_=vps[:, :, 17], func=Exp
                    )
                    load["act"] += _act_cost(8)
                    nc.vector.tensor_tensor(
                        out=v_sb[:, 8 * g : 8 * g + 8, :],
                        in0=vps[:],
                        in1=wexp[:, 8 * g : 8 * g + 8].broadcast_to([C, 8, 32]),
                        op=mybir.AluOpType.mult,
                    )
                    load["dve"] += _dve_cost(256)

                emit_g_slice(0, split=True)
                emit_v_group(0)

                # ---- main loop: flat global schedule, cross-boundary AV lag ----
                NG = 16  # groups (== DR pairs) per chunk

                def emit_sts(c, gi, squad):
                    csl = slice(c * CHUNK, (c + 1) * CHUNK)
                    for t in range(2):
                        j = 2 * gi + t
                        nc.tensor.matmul(
                            squad[:, t, :],
                            x8_sb[:, :, j * 128 : (j + 1) * 128],
                            g8_sb[:, :, csl],
                            start=True, stop=True,
                            perf_mode=DR,
                        )

                def emit_exp(gi, squad, parity=0):
                    j0 = 2 * gi
                    if abs(load["act"] - load["dve"]) > 999999.0:
                        eng = pick_engine()
                    else:
                        eng = "act" if parity == 0 else "dve"
                    if eng == "act":
                        nc.scalar.activation(
                            out=et[:, j0 : j0 + 2, :],
                            in_=squad[:],
                            func=Exp,
                            bias=negshift,
                        )
                        load["act"] += _act_cost(2 * CHUNK)
                    else:
                        nc.vector.tensor_scalar(
                            out=et[:, j0 : j0 + 2, :].bitcast(u8),
                            in0=squad[:],
                            scalar1=A8,
                            scalar2=B8P,
                            op0=mybir.AluOpType.mult,
                            op1=mybir.AluOpType.add,
                        )
                        load["dve"] += _dve_cost(2 * CHUNK)

                def emit_av(acc_c, p):
                    nc.tensor.matmul(
                        acc_c[:],
                        v_sb[:, 2 * p : 2 * p + 2, 0:17],
                        et[:, 2 * p : 2 * p + 2, :],
                        start=(p == 0),
                        stop=(p == NG - 1),
                        perf_mode=DR,
                        skip_group_check=True,
                    )

                def emit_epi_evac(acc_c):
                    acc_sb = accsbp.tile([17, CHUNK], f32, tag="accsb")
                    nc.scalar.copy(acc_sb[:], acc_c[:])
                    load["act"] += _act_cost(CHUNK)
                    return acc_sb

                def emit_epi_norm(acc_sb):
                    tps4 = ps_s.tile([C, 4, 18], f32, tag="squad")
                    for s in range(4):
                        nc.tensor.matmul(
                            tps4[:, s, :],
                            acc_sb[:, s * 128 : (s + 1) * 128],
                            idpad,
                            start=True, stop=True,
                        )
                    recip4 = episb.tile([C, 4], f32, tag="recip")
                    nc.vector.reciprocal(recip4[:], tps4[:, :, 16])
                    load["dve"] += _dve_cost(4)
                    onorm4 = episb.tile([C, 4, 16], bf16, tag="onorm")
                    nc.vector.tensor_tensor(
                        out=onorm4[:],
                        in0=tps4[:, :, 0:16],
                        in1=recip4[:].broadcast_to([C, 4, 16]),
                        op=mybir.AluOpType.mult,
                    )
                    load["dve"] += _dve_cost(64)
                    return onorm4

                def emit_epi_proj(onorm4, c_prev):
                    pps = ps_epi.tile([D, CHUNK], f32, tag="pps")
                    nc.tensor.matmul(
                        pps[:], ones16, bp512,
                        start=True, stop=False, skip_group_check=True,
                    )
                    for s in range(4):
                        nc.tensor.matmul(
                            pps[:, s * 128 : (s + 1) * 128],
                            onorm4[:, s, :],
                            wpbf,
                            start=False, stop=(s == 3), skip_group_check=True,
                        )
                    osb = episb.tile([D, CHUNK], f32, tag="osb")
                    nc.scalar.copy(osb[:], pps[:])
                    load["act"] += _act_cost(CHUNK)
                    nc.sync.dma_start(
                        out=out_d[:, c_prev * CHUNK : (c_prev + 1) * CHUNK],
                        in_=osb[:],
                    )

                AV_LAG = 4
                _pro = {
                    1: lambda: emit_g_slice(2),
                    3: lambda: emit_v_group(1),
                    5: lambda: emit_g_slice(4),
                    7: lambda: emit_v_group(2),
                    9: lambda: emit_g_slice(6),
                    11: lambda: emit_v_group(3),
                }
                accs = {}
                epi = {}  # c -> dict of staged products
                av_next = 0  # global AV emission cursor (over 128 pairs)
                for G in range(NCHUNK * NG):
                    c, gi = divmod(G, NG)
                    squad = ps_s.tile([C, 2, CHUNK], f32, tag="squad")
                    emit_sts(c, gi, squad)
                    emit_exp(gi, squad, parity=G % 2)
                    if c == 0 and gi in _pro:
                        _pro.pop(gi)()
                    # drain AV pairs whose exp is AV_LAG groups back
                    while av_next <= G - AV_LAG:
                        cp, p = divmod(av_next, NG)
                        if p == 0:
                            accs[cp] = ps_acc.tile([17, CHUNK], f32, tag="acc", name=f"acc{cp}")
                        emit_av(accs[cp], p)
                        av_next += 1
                    if gi == 4 and c > 0:
                        epi[c - 1] = {"acc_sb": emit_epi_evac(accs.pop(c - 1))}
                    if gi == 5 and c > 0:
                        epi[c - 1]["onorm"] = emit_epi_norm(epi[c - 1]["acc_sb"])
                    if gi == 9 and c > 0:
                        emit_epi_proj(epi.pop(c - 1)["onorm"], c - 1)
                while av_next < NCHUNK * NG:
                    cp, p = divmod(av_next, NG)
                    if p == 0:
                        accs[cp] = ps_acc.tile([17, CHUNK], f32, tag="acc", name=f"acc{cp}")
                    emit_av(accs[cp], p)
                    av_next += 1
                c_last = NCHUNK - 1
                acc_sb = emit_epi_evac(accs.pop(c_last))
                emit_epi_proj(emit_epi_norm(acc_sb), c_last)

    nc.compile()
    return nc


def _get_program():
    if "nc" not in _CACHE:
        _CACHE["nc"] = _build()
    return _CACHE["nc"]


def _make_in_maps(x, w_qkv, b_qkv, w_proj, b_proj):
    import ml_dtypes

    bf = ml_dtypes.bfloat16
    x_f32 = np.asarray(x, dtype=np.float32).reshape(C, L)
    x_cl = np.ascontiguousarray(x_f32.astype(bf))
    x8 = np.ascontiguousarray(
        (x_f32 * 0.25).reshape(C // 2, 2, L).astype(ml_dtypes.float8_e4m3)
    )
    w_qkv = np.asarray(w_qkv, dtype=np.float32)
    b_qkv = np.asarray(b_qkv, dtype=np.float32)
    w_proj = np.asarray(w_proj, dtype=np.float32)
    b_proj = np.asarray(b_proj, dtype=np.float32)
    wpT = np.ascontiguousarray(w_proj.T)

    cbase = np.zeros((C, CB_W), dtype=np.float32)
    cbase[0:17, 0:17] = np.eye(17, dtype=np.float32)  # idpad (col 17 zero)
    cbase[:, 18] = -SHIFT

    in_maps = []
    for i in range(N_CORES):
        rows_q = np.arange(D) * 24 + i * 3
        Wq = w_qkv[rows_q]
        Wk = w_qkv[rows_q + 1]
        Wv = w_qkv[rows_q + 2]
        bq = b_qkv[rows_q]
        bv = b_qkv[rows_q + 2]

        bbl = np.zeros((C, BB_W), dtype=bf)
        bbl[:, 0:128] = wpT.astype(bf)
        wvb = np.zeros((C, 32), dtype=np.float32)
        wvb[:, 0:16] = Wv.T
        wvb[:, 17] = 4.0 * (Wk.T @ bq)
        bbl[:, 128:160] = wvb.astype(bf)
        bbl[:, 160:288] = (16.0 * (Wq.T @ Wk)).astype(bf)
        bbl[0, 288:800] = np.tile(b_proj, 4).astype(bf)
        bbl[0, 800:816] = np.ones(16, dtype=bf)
        bv32 = np.zeros(32, dtype=np.float32)
        bv32[0:16] = bv
        bv32[16] = 1.0
        bbl[0, 816:848] = bv32.astype(bf)
        bbl[0, 848:976] = np.ones(128, dtype=bf)
        in_maps.append({"x_cl": x_cl, "x8": x8, "cblob": cbase, "bblob": bbl})
    return in_maps


def _run(in_maps, trace=False):
    from concourse.bass_utils import run_bass_kernel_spmd

    nc = _get_program()
    return run_bass_kernel_spmd(nc, in_maps, list(range(N_CORES)), trace=trace)


def _assemble(results):
    out = np.empty((1, C, H, W), dtype=np.float32)
    for i in range(N_CORES):
        out[0, i * D : (i + 1) * D] = results[i]["out"].reshape(D, H, W)
    return out


def kernel(x, w_qkv, b_qkv, w_proj, b_proj):
    in_maps = _make_in_maps(x, w_qkv, b_qkv, w_proj, b_proj)
    r = _run(in_maps, trace=False)
    return _assemble(r.results)


def kernel_with_timing(x, w_qkv, b_qkv, w_proj, b_proj):
    """Like kernel() but also returns an HW execution time estimate in ns."""
    in_maps = _make_in_maps(x, w_qkv, b_qkv, w_proj, b_proj)
    try:
        r = _run(in_maps, trace=True)
        exec_ns = r.exec_time_ns
    except ModuleNotFoundError:
        r = _run(in_maps, trace=False)
        exec_ns = None
    if exec_ns is None:
        exec_ns = _CACHE.get("tlsim_ns")
        if exec_ns is None:
            from concourse.timeline_sim import TimelineSim

            exec_ns = int(TimelineSim(_get_program()).simulate())
            _CACHE["tlsim_ns"] = exec_ns
    return _assemble(r.results), exec_ns


# revision 23
# speedup vs baseline: 1.0357x; 1.0108x over previous
"""Trainium2 Bass kernel for nn_AttnBlock (B=1, C=128, H=32, W=128, 8 heads).

Sharding: one attention head per NeuronCore (8 heads / 8 cores). Each core
computes its head's attention over L=4096 positions and the final W-axis
projection for its 16-channel output slab. Host gathers 8 slabs.

v2 design (vs the flash baseline):
  * S^T via the rank-16 factor-through-weights trick: G = (4*Wk^T Wq) @ x is
    computed once on-device (128-deep contractions), then every S^T tile is
    x_tile^T @ G_chunk -- no q/k tensors, one evacuation (G) instead of two.
  * q-bias folded EXACTLY into a per-key reweighting of V: softmax(q_i.k_j +
    bq.k_j + const_i) => multiply [v_j|1] by w_j = exp(4*bq.k_j). The bias
    row 4*bq.k_j is produced as an extra column of the v matmul and exp'd on
    ScalarE; k-bias and bq.bk terms cancel in softmax exactly.
  * exp tiles (128 x 1536) split between ScalarE (exact exp -> fp8e4) and
    VectorE (Schraudolph int-bit trick -> uint8 saturating -> fp8e4 bits;
    negative-bit underflow saturates to 0.0 which is the correct flush).
  * A@V in fp8e4 with perf_mode=DoubleRow: one matmul contracts TWO j-tiles
    ([128, 2, 17] weights x [128, 2, 512] moving), with the softmax
    denominator as a 17th weight column (ones*w_j).
  * epilogue: transpose via idmatmul, batched reciprocal + broadcast-multiply
    normalize, W-axis projection in bf16.
"""

import math as _math

import numpy as np

N_CORES = 8
C = 128
H = 32
W = 128
L = H * W  # 4096
F = 8  # heads
D = 16  # head dim
CHUNK = 512
NCHUNK = L // CHUNK  # 8
NJT = L // 128  # 32 j-tiles
SHIFT = 2.5  # global exp shift for fp8 range (cancels in softmax)
A8 = 8.0 / _math.log(2.0)  # Schraudolph scale for e4m3 bits
B8P = (56.0 - 0.5) - SHIFT * A8  # magic + shift folded
CB_W = 20  # f32 cblob: idpad (17,18) | negshift col
# bf16 blob: wpbf 0:128 | wvb 128:160 | gw 160:288 | row0: bp512 288:800,
# ones16 800:816, bv32 816:848, ones128 848:976
BB_W = 976

# cost-model constants for build-time ACT/DVE load balancing (ns)
_ACT_CY = 1e9 / 1.2e9
_DVE_CY = 1e9 / 0.96e9


def _act_cost(fd):
    return (fd + 222) * _ACT_CY


def _dve_cost(fd, psum=True):
    return (fd + (120 if psum else 58)) * _DVE_CY


_CACHE = {}


def _build():
    import concourse.tile as tile
    from concourse import bacc, mybir

    f32 = mybir.dt.float32
    bf16 = mybir.dt.bfloat16
    fp8 = mybir.dt.float8e4
    u8 = mybir.dt.uint8
    Exp = mybir.ActivationFunctionType.Exp
    DR = mybir.MatmulPerfMode.DoubleRow

    nc = bacc.Bacc("TRN2", target_bir_lowering=False, debug=False)

    x_d = nc.dram_tensor("x_cl", [C, L], bf16, kind="ExternalInput").ap()
    x8_d = nc.dram_tensor("x8", [C // 2, 2, L], fp8, kind="ExternalInput").ap()
    cb_d = nc.dram_tensor("cblob", [C, CB_W], f32, kind="ExternalInput").ap()
    bb_d = nc.dram_tensor("bblob", [C, BB_W], bf16, kind="ExternalInput").ap()
    out_d = nc.dram_tensor("out", [D, L], f32, kind="ExternalOutput").ap()

    # build-time engine load (ns) for balancing flexible work
    load = {"act": 1283.0, "dve": 0.0}  # act table load charged up front

    def pick_engine():
        return "act" if load["act"] <= load["dve"] else "dve"

    with tile.TileContext(nc) as tc:
        with (
            tc.tile_pool(name="consts", bufs=1) as consts,
            tc.tile_pool(name="accsb", bufs=2) as accsbp,
            tc.tile_pool(name="episb", bufs=4) as episb,
        ):
            cb = consts.tile([C, CB_W], f32)
            idpad = cb[0:17, 0:18]
            negshift = cb[:, 18:19]
            bb = consts.tile([C, BB_W], bf16)
            wpbf = bb[:, 0:128]
            wvb = bb[:, 128:160]
            gw = bb[:, 160:288]
            bp512 = bb[0:1, 288:800]
            ones16 = bb[0:1, 800:816]
            bv32 = bb[0:1, 816:848]
            ones128row = bb[0:1, 848:976]

            x_sb = consts.tile([C, L], bf16)

            def dma_x(cch, q):
                q.dma_start(
                    out=x_sb[:, cch * CHUNK : (cch + 1) * CHUNK],
                    in_=x_d[:, cch * CHUNK : (cch + 1) * CHUNK],
                )

            x8_sb = consts.tile([C // 2, 2, L], fp8)
            g_f8 = consts.tile([C, L], fp8)
            g8_sb = consts.tile([C // 2, 2, L], fp8)

            # DMA orchestration, critical-path first.  NOTHING on the
            # scalar queue (a DMA there blocks the ACT sequencer ~667ns);
            # sync (HWDGE) leads with the chain bb/x0/cb/x8p0 that gates
            # the first S^T group; bulk goes to the Pool SWDGE.
            def dma_x8(piece, q):
                q.dma_start(
                    out=x8_sb[:, :, piece * 1024 : (piece + 1) * 1024],
                    in_=x8_d[:, :, piece * 1024 : (piece + 1) * 1024],
                )

            nc.sync.dma_start(out=bb, in_=bb_d)
            dma_x(0, nc.sync)
            nc.sync.dma_start(out=cb, in_=cb_d)
            dma_x8(0, nc.sync)
            dma_x(1, nc.gpsimd)
            dma_x(2, nc.gpsimd)
            dma_x(3, nc.gpsimd)
            dma_x8(1, nc.gpsimd)
            dma_x(5, nc.gpsimd)
            dma_x8(2, nc.gpsimd)
            dma_x(7, nc.gpsimd)
            dma_x8(3, nc.gpsimd)
            et = consts.tile([C, NJT, CHUNK], fp8)
            v_sb = consts.tile([C, NJT, 32], fp8)
            wexp = consts.tile([C, NJT], f32)

            with (
                tc.tile_pool(name="ps_s", bufs=3, space="PSUM") as ps_s,
                tc.tile_pool(name="ps_acc", bufs=1, space="PSUM") as ps_acc,
                tc.tile_pool(name="ps_epi", bufs=1, space="PSUM") as ps_epi,
            ):
                # warm the ACT exp table immediately (no DMA dependency)
                dummy = episb.tile([1, 2], f32, tag="dummy")
                nc.gpsimd.memset(dummy[:], 0.5)
                nc.scalar.activation(out=dummy[:], in_=dummy[:], func=Exp)

                # ---- G = (4 Wk^T Wq) @ x and v-tile helpers; slice 0
                # upfront, the rest interleaved into chunk 0's schedule ----
                def emit_g_slice(s0, split=False):
                    # g8[u, e, i] = g_f8[2u + e, i] (partition-pair permute
                    # via DMA on the otherwise idle DMA track)
                    gps = ps_s.tile([C, 2, CHUNK], f32, tag="squad", name=f"gps{s0}")
                    sl01 = slice(s0 * CHUNK, (s0 + 2) * CHUNK)
                    for t in range(2):
                        sl = slice((s0 + t) * CHUNK, (s0 + t + 1) * CHUNK)
                        nc.tensor.matmul(
                            gps[:, t, :], gw, x_sb[:, sl], start=True, stop=True
                        )
                        if split:
                            nc.scalar.copy(g_f8[:, sl], gps[:, t, :])
                            load["act"] += _act_cost(CHUNK)
                            for e in range(2):
                                nc.sync.dma_start(
                                    out=g8_sb[:, e, sl], in_=g_f8[e : C : 2, sl]
                                )
                    if not split:
                        nc.scalar.copy(g_f8[:, sl01], gps[:])
                        load["act"] += _act_cost(2 * CHUNK)
                        for e in range(2):
                            nc.sync.dma_start(
                                out=g8_sb[:, e, sl01], in_=g_f8[e : C : 2, sl01]
                            )

                def emit_v_group(g):
                    vps = ps_s.tile([C, 8, 32], f32, tag="squad", name=f"vps{g}")
                    for u in range(8):
                        t = 8 * g + u
                        nc.tensor.matmul(
                            vps[:, u, :], ones128row, bv32,
                            start=True, stop=False, skip_group_check=True,
                        )
                        nc.tensor.matmul(
                            vps[:, u, :], x_sb[:, t * 128 : (t + 1) * 128], wvb,
                            start=False, stop=True, skip_group_check=True,
                        )
                    nc.scalar.activation(
                        out=wexp[:, 8 * g : 8 * g + 8], in_=vps[:, :, 17], func=Exp
                    )
                    load["act"] += _act_cost(8)
                    nc.vector.tensor_tensor(
                        out=v_sb[:, 8 * g : 8 * g + 8, :],
                        in0=vps[:],
                        in1=wexp[:, 8 * g : 8 * g + 8].broadcast_to([C, 8, 32]),
                        op=mybir.AluOpType.mult,
                    )
                    load["dve"] += _dve_cost(256)

                emit_g_slice(0, split=True)
                dma_x(4, nc.sync)
                dma_x(6, nc.sync)
                emit_v_group(0)

                # ---- main loop: flat global schedule, cross-boundary AV lag ----
                NG = 16  # groups (== DR pairs) per chunk

                def emit_sts(c, gi, squad):
                    csl = slice(c * CHUNK, (c + 1) * CHUNK)
                    for t in range(2):
                        j = 2 * gi + t
                        nc.tensor.matmul(
                            squad[:, t, :],
                            x8_sb[:, :, j * 128 : (j + 1) * 128],
                            g8_sb[:, :, csl],
                            start=True, stop=True,
                            perf_mode=DR,
                        )

                def emit_exp(gi, squad, parity=0):
                    j0 = 2 * gi
                    if abs(load["act"] - load["dve"]) > 999999.0:
                        eng = pick_engine()
                    else:
                        eng = "act" if parity == 0 else "dve"
                    if eng == "act":
                        nc.scalar.activation(
                            out=et[:, j0 : j0 + 2, :],
                            in_=squad[:],
                            func=Exp,
                            bias=negshift,
                        )
                        load["act"] += _act_cost(2 * CHUNK)
                    else:
                        nc.vector.tensor_scalar(
                            out=et[:, j0 : j0 + 2, :].bitcast(u8),
                            in0=squad[:],
                            scalar1=A8,
                            scalar2=B8P,
                            op0=mybir.AluOpType.mult,
                            op1=mybir.AluOpType.add,
                        )
                        load["dve"] += _dve_cost(2 * CHUNK)

                def emit_av(acc_c, p):
                    nc.tensor.matmul(
                        acc_c[:],
                        v_sb[:, 2 * p : 2 * p + 2, 0:17],
                        et[:, 2 * p : 2 * p + 2, :],
                        start=(p == 0),
                        stop=(p == NG - 1),
                        perf_mode=DR,
                        skip_group_check=True,
                    )

                def emit_epi_evac(acc_c):
                    acc_sb = accsbp.tile([17, CHUNK], f32, tag="accsb")
                    nc.scalar.copy(acc_sb[:], acc_c[:])
                    load["act"] += _act_cost(CHUNK)
                    return acc_sb

                def emit_epi_norm(acc_sb):
                    tps4 = ps_s.tile([C, 4, 18], f32, tag="squad")
                    for s in range(4):
                        nc.tensor.matmul(
                            tps4[:, s, :],
                            acc_sb[:, s * 128 : (s + 1) * 128],
                            idpad,
                            start=True, stop=True,
                        )
                    recip4 = episb.tile([C, 4], f32, tag="recip")
                    nc.vector.reciprocal(recip4[:], tps4[:, :, 16])
                    load["dve"] += _dve_cost(4)
                    onorm4 = episb.tile([C, 4, 16], bf16, tag="onorm")
                    nc.vector.tensor_tensor(
                        out=onorm4[:],
                        in0=tps4[:, :, 0:16],
                        in1=recip4[:].broadcast_to([C, 4, 16]),
                        op=mybir.AluOpType.mult,
                    )
                    load["dve"] += _dve_cost(64)
                    return onorm4

                def emit_epi_proj(onorm4, c_prev):
                    pps = ps_epi.tile([D, CHUNK], f32, tag="pps")
                    nc.tensor.matmul(
                        pps[:], ones16, bp512,
                        start=True, stop=False, skip_group_check=True,
                    )
                    for s in range(4):
                        nc.tensor.matmul(
                            pps[:, s * 128 : (s + 1) * 128],
                            onorm4[:, s, :],
                            wpbf,
                            start=False, stop=(s == 3), skip_group_check=True,
                        )
                    osb = episb.tile([D, CHUNK], f32, tag="osb")
                    nc.scalar.copy(osb[:], pps[:])
                    load["act"] += _act_cost(CHUNK)
                    nc.sync.dma_start(
                        out=out_d[:, c_prev * CHUNK : (c_prev + 1) * CHUNK],
                        in_=osb[:],
                    )

                AV_LAG = 4
                _pro = {
                    1: lambda: emit_g_slice(2),
                    3: lambda: emit_v_group(1),
                    5: lambda: emit_g_slice(4),
                    7: lambda: emit_v_group(2),
                    9: lambda: emit_g_slice(6),
                    11: lambda: emit_v_group(3),
                }
                accs = {}
                epi = {}  # c -> dict of staged products
                av_next = 0  # global AV emission cursor (over 128 pairs)
                for G in range(NCHUNK * NG):
                    c, gi = divmod(G, NG)
                    squad = ps_s.tile([C, 2, CHUNK], f32, tag="squad")
                    emit_sts(c, gi, squad)
                    emit_exp(gi, squad, parity=G % 2)
                    if c == 0 and gi in _pro:
                        _pro.pop(gi)()
                    # drain AV pairs whose exp is AV_LAG groups back
                    while av_next <= G - AV_LAG:
                        cp, p = divmod(av_next, NG)
                        if p == 0:
                            accs[cp] = ps_acc.tile([17, CHUNK], f32, tag="acc", name=f"acc{cp}")
                        emit_av(accs[cp], p)
                        av_next += 1
                    if gi == 4 and c > 0:
                        epi[c - 1] = {"acc_sb": emit_epi_evac(accs.pop(c - 1))}
                    if gi == 5 and c > 0:
                        epi[c - 1]["onorm"] = emit_epi_norm(epi[c - 1]["acc_sb"])
                    if gi == 9 and c > 0:
                        emit_epi_proj(epi.pop(c - 1)["onorm"], c - 1)
                while av_next < NCHUNK * NG:
                    cp, p = divmod(av_next, NG)
                    if p == 0:
                        accs[cp] = ps_acc.tile([17, CHUNK], f32, tag="acc", name=f"acc{cp}")
                    emit_av(accs[cp], p)
                    av_next += 1
                c_last = NCHUNK - 1
                acc_sb = emit_epi_evac(accs.pop(c_last))
                emit_epi_proj(emit_epi_norm(acc_sb), c_last)

    nc.compile()
    return nc


def _get_program():
    if "nc" not in _CACHE:
        _CACHE["nc"] = _build()
    return _CACHE["nc"]


def _make_in_maps(x, w_qkv, b_qkv, w_proj, b_proj):
    import ml_dtypes

    bf = ml_dtypes.bfloat16
    x_f32 = np.asarray(x, dtype=np.float32).reshape(C, L)
    x_cl = np.ascontiguousarray(x_f32.astype(bf))
    x8 = np.ascontiguousarray(
        (x_f32 * 0.25).reshape(C // 2, 2, L).astype(ml_dtypes.float8_e4m3)
    )
    w_qkv = np.asarray(w_qkv, dtype=np.float32)
    b_qkv = np.asarray(b_qkv, dtype=np.float32)
    w_proj = np.asarray(w_proj, dtype=np.float32)
    b_proj = np.asarray(b_proj, dtype=np.float32)
    wpT = np.ascontiguousarray(w_proj.T)

    cbase = np.zeros((C, CB_W), dtype=np.float32)
    cbase[0:17, 0:17] = np.eye(17, dtype=np.float32)  # idpad (col 17 zero)
    cbase[:, 18] = -SHIFT

    in_maps = []
    for i in range(N_CORES):
        rows_q = np.arange(D) * 24 + i * 3
        Wq = w_qkv[rows_q]
        Wk = w_qkv[rows_q + 1]
        Wv = w_qkv[rows_q + 2]
        bq = b_qkv[rows_q]
        bv = b_qkv[rows_q + 2]

        bbl = np.zeros((C, BB_W), dtype=bf)
        bbl[:, 0:128] = wpT.astype(bf)
        wvb = np.zeros((C, 32), dtype=np.float32)
        wvb[:, 0:16] = Wv.T
        wvb[:, 17] = 4.0 * (Wk.T @ bq)
        bbl[:, 128:160] = wvb.astype(bf)
        bbl[:, 160:288] = (16.0 * (Wq.T @ Wk)).astype(bf)
        bbl[0, 288:800] = np.tile(b_proj, 4).astype(bf)
        bbl[0, 800:816] = np.ones(16, dtype=bf)
        bv32 = np.zeros(32, dtype=np.float32)
        bv32[0:16] = bv
        bv32[16] = 1.0
        bbl[0, 816:848] = bv32.astype(bf)
        bbl[0, 848:976] = np.ones(128, dtype=bf)
        in_maps.append({"x_cl": x_cl, "x8": x8, "cblob": cbase, "bblob": bbl})
    return in_maps


def _run(in_maps, trace=False):
    from concourse.bass_utils import run_bass_kernel_spmd

    nc = _get_program()
    return run_bass_kernel_spmd(nc, in_maps, list(range(N_CORES)), trace=trace)


def _assemble(results):
    out = np.empty((1, C, H, W), dtype=np.float32)
    for i in range(N_CORES):
        out[0, i * D : (i + 1) * D] = results[i]["out"].reshape(D, H, W)
    return out


def kernel(x, w_qkv, b_qkv, w_proj, b_proj):
    in_maps = _make_in_maps(x, w_qkv, b_qkv, w_proj, b_proj)
    r = _run(in_maps, trace=False)
    return _assemble(r.results)


def kernel_with_timing(x, w_qkv, b_qkv, w_proj, b_proj):
    """Like kernel() but also returns an HW execution time estimate in ns."""
    in_maps = _make_in_maps(x, w_qkv, b_qkv, w_proj, b_proj)
    try:
        r = _run(in_maps, trace=True)
        exec_ns = r.exec_time_ns
    except ModuleNotFoundError:
        r = _run(in_maps, trace=False)
        exec_ns = None
    if exec_ns is None:
        exec_ns = _CACHE.get("tlsim_ns")
        if exec_ns is None:
            from concourse.timeline_sim import TimelineSim

            exec_ns = int(TimelineSim(_get_program()).simulate())
            _CACHE["tlsim_ns"] = exec_ns
    return _assemble(r.results), exec_ns
